# revision 3
# baseline (speedup 1.0000x reference)
"""Multi-head attention kernel for Trainium2 (Bass/Tile), 8-core SPMD.

Problem: x[2, 2048, 1024], 16 heads x 64 dims, boolean key mask (all ones
per spec), W_qkv[1024, 3072], W_out[1024, 1024]. Reference is fp32.

Sharding: core c -> (batch b = c // 4, head-group g = c % 4 of 4 heads).
Each core computes attention for its 4 heads of its batch and a partial
output projection [2048, 1024]; the host sums the 4 head-group partials
per batch and adds b_out plus the V-bias correction.

Engine roofline per core (fp16): PE ~137us of matmul stream, ScalarE
~143us of softmax exp ([128,1024] ACTIVATE per kchunk) -- the exp stream
is the pacing engine, so the kernel is organized to keep it saturated:

  - inputs are host-prepacked so every DMA is contiguous per partition
    (large packets), split across the two HWDGE queues (sync + scalar)
    in first-use order; only K-pair0/Q-pair0 of strip 0 gate the first
    score matmul (~8us in).
  - groups run p-major: (s0..s3, pair0) then (s0..s3, pair1). All other
    projections (remaining K/Q strips, V chunks) are woven into k-loop
    slots as background PE tasks with deadlines ahead of first use.
  - scoresT pair [kpos 128, qpos 1024] via two concurrent row-tiled
    cont-64 matmuls; exp as one ACT op per kchunk (bias = mask bias);
    AV lags one kchunk (lhsT = V chunk [128, 64+1 ones col] so the
    softmax denominator rides the AV matmul).
  - normalization: approx-reciprocal of the denominator row (DVE), a
    1-partition PE outer product broadcasts it to [64, 512] PSUM, DVE
    multiply writes normalized outT. No GpSimd in the chain.
  - outproj for strip s runs as background tasks one full group after
    (s, pair1) so nothing near a group boundary waits on the normalize
    chain (avoids head-of-line blocking in the in-order PE queue).
  - y is written fp16 (partials are summed on the host in fp32).
"""

import sys

sys.path.insert(0, "/opt/trn_rl_repo")

import numpy as np

B, N, D = 2, 2048, 1024
HEADS, DH = 16, 64
SCALE = DH ** -0.5
NCORES = 8
GROUPS = 4                      # head groups (tensor parallel)
DLOC = (HEADS // GROUPS) * DH   # 256 local inner dims per core

QC = 4                          # 512-wide query strips
DC = 8                          # contraction chunks
KC = 16                         # 128-wide key chunks

_CACHE = {}


def build_model(with_bias=False):
    """Build (once) the single-core Bass/Tile program shared by all 8 cores.

    with_bias adds the Q/K bias matmuls (b_qkv is all-zero per the problem
    spec, so the default model omits them)."""
    key = ("nc", with_bias)
    if key in _CACHE:
        return _CACHE[key]

    from concourse import bacc, mybir, tile

    f32 = mybir.dt.float32
    f16 = mybir.dt.float16
    AF = mybir.ActivationFunctionType

    nc = bacc.Bacc("TRN2", target_bir_lowering=False, debug=False)

    xt_d = nc.dram_tensor("xt", [128, QC, DC, 512], f16, kind="ExternalInput").ap()
    wqkv_d = nc.dram_tensor("wqkv", [128, 3, DC, DLOC], f16, kind="ExternalInput").ap()
    wout_d = nc.dram_tensor("wout", [128, 2, D], f16, kind="ExternalInput").ap()
    mb_d = nc.dram_tensor("mb", [128, KC], f32, kind="ExternalInput").ap()
    onesf_d = nc.dram_tensor("onesf", [1, 64], f32, kind="ExternalInput").ap()
    ones_d = nc.dram_tensor("ones16", [128, 128], f16, kind="ExternalInput").ap()
    brow_d = nc.dram_tensor("brow", [1, 3 * DLOC], f16, kind="ExternalInput").ap()
    y_d = nc.dram_tensor("y", [N, D], f16, kind="ExternalOutput").ap()

    with tile.TileContext(nc) as tc:
        with (
            tc.tile_pool(name="res", bufs=1) as res,
            tc.tile_pool(name="exp", bufs=8) as exp_pool,
            tc.tile_pool(name="ysb", bufs=3) as y_pool,
            tc.tile_pool(name="small", bufs=8) as small_pool,
            tc.tile_pool(name="ps", bufs=4, space="PSUM") as ps,
            tc.tile_pool(name="spair", bufs=2, space="PSUM") as ps_s,
        ):
            xt = res.tile([128, QC, DC, 512], f16)
            wqkv = res.tile([128, 3, DC, DLOC], f16)
            wout = res.tile([128, 2, D], f16)
            mb = res.tile([128, KC], f32)
            onesf = res.tile([1, 64], f32)
            qt = res.tile([128, 2, N], f16)
            kt = res.tile([128, 2, N], f16)
            vn = res.tile([128, KC, 4, 65], f16)
            outt = res.tile([128, 2, N], f16)
            ones16 = res.tile([1, 512], f16) if with_bias else None
            brow = res.tile([1, 3 * DLOC], f16) if with_bias else None

            # ---- input DMAs. sync queue: K weights then xt strips (the
            # critical path for the projection stream). scalar queue: the rest,
            # in first-use order; it is idle until the first exp (~8us).
            nc.sync.dma_start(wqkv[:, 1:2], wqkv_d[:, 1:2])
            for s in range(QC):
                nc.sync.dma_start(xt[:, s:s + 1], xt_d[:, s:s + 1])
            nc.scalar.dma_start(mb[:], mb_d[:])
            nc.scalar.dma_start(wqkv[:, 0:1], wqkv_d[:, 0:1])
            nc.scalar.dma_start(onesf[:], onesf_d[:])
            nc.scalar.dma_start(wqkv[:, 2:3], wqkv_d[:, 2:3])
            nc.scalar.dma_start(
                vn[:, :, :, 64:65],
                ones_d[:, 0:64].rearrange("p (j h) -> p j h", h=4).unsqueeze(-1),
            )
            nc.scalar.dma_start(wout[:], wout_d[:])
            if with_bias:
                nc.scalar.dma_start(
                    ones16[:],
                    ones_d.rearrange("a b -> (a b)")[0:512].unsqueeze(0),
                )
                nc.scalar.dma_start(brow[:], brow_d[:])

            # ---- background PE task units (each <= ~0.7us of PE stream) ----
            def qk_mms(t, s, pr, psum, cs):
                """Score-side projection chunk matmuls (t: 0=Q, 1=K)."""
                for c in cs:
                    nc.tensor.matmul(
                        psum[:],
                        wqkv[:, t, c, pr * 128:(pr + 1) * 128],
                        xt[:, s, c, :],
                        start=(c == 0),
                        stop=(not with_bias and c == DC - 1),
                    )
                if with_bias and DC - 1 in cs:
                    col0 = t * DLOC + pr * 128
                    nc.tensor.matmul(
                        psum[:],
                        brow[0:1, col0:col0 + 128],
                        ones16[0:1, 0:512],
                        start=False,
                        stop=True,
                    )

            def proj_qk_units(t, s, pr, dst):
                """3 units: one strip/pair of the Q^T or K^T projection."""
                state = {}

                def u1():
                    state["ps"] = ps.tile([128, 512], f32, tag="ps", name="qkps")
                    qk_mms(t, s, pr, state["ps"], (0, 1, 2))

                def u2():
                    qk_mms(t, s, pr, state["ps"], (3, 4, 5))

                def u3():
                    qk_mms(t, s, pr, state["ps"], (6, 7))
                    nc.vector.tensor_copy(
                        dst[:, pr, s * 512:(s + 1) * 512], state["ps"][:]
                    )

                return [u1, u2, u3]

            def vproj_unit(j, pr):
                """V projection of seq chunk j for head pair pr (128 cols)."""

                def u():
                    psum = ps.tile([128, 128], f32, tag="ps", name="vps")
                    for c in range(DC):
                        nc.tensor.matmul(
                            psum[:],
                            xt[:, j // 4, c, (j % 4) * 128:(j % 4) * 128 + 128],
                            wqkv[:, 2, c, pr * 128:(pr + 1) * 128],
                            start=(c == 0),
                            stop=(c == DC - 1),
                        )
                    nc.vector.tensor_copy(
                        vn[:, j, 2 * pr:2 * pr + 2, 0:64],
                        psum[:].rearrange("a (h x) -> a h x", h=2),
                    )

                return u

            def outproj_tasks(s):
                """8 units: output projection of strip s as (jj, nb) groups."""
                state = {}
                tasks = []
                for jj in range(4):
                    for nb in range(2):
                        def t(jj=jj, nb=nb):
                            q0 = s * 512 + jj * 128
                            if nb == 0:
                                state[jj] = y_pool.tile(
                                    [128, D], f16, tag="ysb", name="ysb"
                                )
                            ysb = state[jj]
                            yps = ps.tile([128, 512], f32, tag="ps", name="yps")
                            for p_ in range(2):
                                nc.tensor.matmul(
                                    yps[:],
                                    outt[:, p_, q0:q0 + 128],
                                    wout[:, p_, nb * 512:(nb + 1) * 512],
                                    start=(p_ == 0),
                                    stop=(p_ == 1),
                                )
                            nc.vector.tensor_copy(
                                ysb[:, nb * 512:(nb + 1) * 512], yps[:]
                            )
                            if nb == 1:
                                nc.sync.dma_start(y_d[q0:q0 + 128, :], ysb[:])
                        tasks.append(t)
                return tasks

            def attn_group(s, p, slot_tasks):
                """One (strip, head-pair) attention group: 16 kchunk slots of
                scores pair -> exp -> lag-1 AV, with background units woven in,
                then the reciprocal/broadcast/multiply normalization."""
                av = [
                    ps.tile([65, 512], f32, tag="ps", name=f"av{i}")
                    for i in range(2)
                ]
                exs = [None] * KC
                for k in range(KC):
                    sc = ps_s.tile([128, 1024], f32, tag="spair", name="sc")
                    for i in range(2):
                        nc.tensor.matmul(
                            sc[:, i * 512:(i + 1) * 512],
                            kt[64 * i:64 * i + 64, p, k * 128:(k + 1) * 128],
                            qt[64 * i:64 * i + 64, p, s * 512:(s + 1) * 512],
                            start=True,
                            stop=True,
                        )
                    ex = exp_pool.tile([128, 1024], f16, tag="exp", name="ex")
                    nc.scalar.activation(
                        ex[:], sc[:], AF.Exp, bias=mb[:, k:k + 1], scale=1.0
                    )
                    exs[k] = ex
                    if k > 0:
                        for i in range(2):
                            nc.tensor.matmul(
                                av[i][:],
                                vn[:, k - 1, 2 * p + i, :],
                                exs[k - 1][:, i * 512:(i + 1) * 512],
                                start=(k - 1 == 0),
                                stop=False,
                            )
                    for u in slot_tasks.get(k, ()):
                        u()
                for i in range(2):
                    nc.tensor.matmul(
                        av[i][:],
                        vn[:, KC - 1, 2 * p + i, :],
                        exs[KC - 1][:, i * 512:(i + 1) * 512],
                        start=False,
                        stop=True,
                    )
                # normalize: stash unnormalized rows (frees av psum), approx
                # reciprocal of the denominator row, broadcast via a
                # 1-partition PE outer product, DVE multiply into outT.
                for i in range(2):
                    dnr = small_pool.tile([1, 512], f32, tag="dnr", name="dnr")
                    nc.vector.tensor_copy(dnr[:], av[i][64:65, :])
                    un = small_pool.tile([64, 512], f32, tag="un", name="un")
                    nc.vector.tensor_copy(un[:], av[i][0:64, :])
                    rc = small_pool.tile([1, 512], f32, tag="rc", name="rc")
                    nc.vector.reciprocal_approx_fast(rc[:], dnr[:])
                    bc = ps.tile([64, 512], f32, tag="ps", name="bc")
                    nc.tensor.matmul(
                        bc[:], onesf[0:1, 0:64], rc[0:1, :], start=True, stop=True
                    )
                    nc.vector.tensor_mul(
                        outt[64 * i:64 * i + 64, p, s * 512:(s + 1) * 512],
                        un[:],
                        bc[:],
                    )

            def sched(assignments):
                """{slot: [units...]} from a list of (slot, unit)."""
                d = {}
                for sl, u in assignments:
                    d.setdefault(sl, []).append(u)
                return d

            # ---- phase 0: minimal pre-attention (K and Q strip 0, pair 0)
            for u in proj_qk_units(1, 0, 0, kt):
                u()
            for u in proj_qk_units(0, 0, 0, qt):
                u()

            # ---- group schedule (p-major) with background weave.
            # deadlines: K(s,0) by group0 slot 4s; vp(j,0) by group0 slot j+1;
            # Q(s,0) before group s; K/Q/v pair1 before group 4(+s).
            asg = {g: [] for g in range(8)}
            for j in range(KC):
                asg[0].append((j, vproj_unit(j, 0)))
            for si, base in ((1, 0), (2, 4), (3, 8)):
                for ui, u in enumerate(proj_qk_units(1, si, 0, kt)):
                    asg[0].append((base + ui, u))
            for ui, u in enumerate(proj_qk_units(0, 1, 0, qt)):
                asg[0].append((12 + ui, u))

            g1 = (proj_qk_units(0, 2, 0, qt) + proj_qk_units(1, 0, 1, kt)
                  + [vproj_unit(j, 1) for j in range(0, 4)])
            g2 = (proj_qk_units(0, 3, 0, qt) + proj_qk_units(1, 1, 1, kt)
                  + [vproj_unit(j, 1) for j in range(4, 9)])
            g3 = (proj_qk_units(0, 0, 1, qt) + proj_qk_units(1, 2, 1, kt)
                  + [vproj_unit(j, 1) for j in range(9, 14)])
            g4 = (proj_qk_units(1, 3, 1, kt) + proj_qk_units(0, 1, 1, qt)
                  + [vproj_unit(j, 1) for j in range(14, 16)])
            g5 = proj_qk_units(0, 2, 1, qt) + outproj_tasks(0)
            g6 = proj_qk_units(0, 3, 1, qt) + outproj_tasks(1)
            g7 = outproj_tasks(2)
            for g, units in ((1, g1), (2, g2), (3, g3), (4, g4),
                             (5, g5), (6, g6), (7, g7)):
                for ui, u in enumerate(units):
                    asg[g].append((ui, u))

            order = [(s, 0) for s in range(QC)] + [(s, 1) for s in range(QC)]
            for g, (s, p) in enumerate(order):
                attn_group(s, p, sched(asg[g]))
            for t in outproj_tasks(3):
                t()

    nc.compile()
    _CACHE[key] = nc
    return nc


def make_in_maps(x, mask, W_qkv, b_qkv, W_out):
    x = np.asarray(x, np.float32)
    W_qkv = np.asarray(W_qkv, np.float32)
    b_qkv = np.asarray(b_qkv, np.float32)
    W_out = np.asarray(W_out, np.float32)
    if mask is None:
        m = np.ones((B, N), bool)
    else:
        mask = np.asarray(mask, bool)
        m = np.concatenate([np.ones((B, 1), bool), mask], axis=1)
    mbias = np.where(m, np.float32(0.0), np.float32(-1e30)).astype(np.float32)

    in_maps = []
    for c in range(NCORES):
        b, g = divmod(c, GROUPS)
        cs = slice(DLOC * g, DLOC * g + DLOC)
        wq = W_qkv[:, 0:D][:, cs] * SCALE
        wk = W_qkv[:, D:2 * D][:, cs]
        wv = W_qkv[:, 2 * D:3 * D][:, cs]
        bq = b_qkv[0:D][cs] * SCALE
        bk = b_qkv[D:2 * D][cs]
        bv = np.zeros(DLOC, np.float32)   # V bias applied in combine()
        # xt[p, s, c, n] = x[b, s*512+n, c*128+p]
        xt = x[b].reshape(QC, 512, DC, 128).transpose(3, 0, 2, 1)
        # wqkv[p, t, c, j] = W_t[c*128+p, j]
        wqkv = np.stack(
            [w.reshape(DC, 128, DLOC).transpose(1, 0, 2) for w in (wq, wk, wv)],
            axis=1,
        )
        in_maps.append({
            "xt": np.ascontiguousarray(xt).astype(np.float16),
            "wqkv": np.ascontiguousarray(wqkv).astype(np.float16),
            "wout": np.ascontiguousarray(
                W_out[cs, :].reshape(2, 128, D).transpose(1, 0, 2)
            ).astype(np.float16),
            "mb": np.ascontiguousarray(mbias[b].reshape(KC, 128).T),
            "onesf": np.ones((1, 64), np.float32),
            "ones16": np.ones((128, 128), np.float16),
            "brow": np.concatenate([bq, bk, bv])[None, :].astype(np.float16),
        })
    return in_maps


def combine(results, b_qkv, W_out, b_out):
    out = np.zeros((B, N, D), np.float32)
    for c in range(NCORES):
        out[c // GROUPS] += np.asarray(results[c]["y"], np.float32)
    b_qkv = np.asarray(b_qkv, np.float32)
    W_out = np.asarray(W_out, np.float32)
    # attention rows sum to 1 -> V bias contributes b_v @ W_out everywhere
    out += (b_qkv[2 * D:3 * D] @ W_out)[None, None, :]
    out += np.asarray(b_out, np.float32)[None, None, :]
    return out


def kernel(x, mask=None, W_qkv=None, b_qkv=None, W_out=None, b_out=None, **kw):
    from concourse.bass_utils import run_bass_kernel_spmd

    qk_bias = np.any(np.asarray(b_qkv, np.float32)[0:2 * D])
    nc = build_model(with_bias=bool(qk_bias))
    in_maps = make_in_maps(x, mask, W_qkv, b_qkv, W_out)
    res = run_bass_kernel_spmd(nc, in_maps, core_ids=list(range(NCORES)))
    return combine(res.results, b_qkv, W_out, b_out)


# revision 5
# speedup vs baseline: 1.2583x; 1.2583x over previous
"""Multi-head attention kernel for Trainium2 (Bass/Tile), 8-core SPMD.

Problem: x[2, 2048, 1024], 16 heads x 64 dims, boolean key mask (all ones
per spec), W_qkv[1024, 3072], W_out[1024, 1024]. Reference is fp32.

Sharding: core c -> (batch b = c // 4, head-group g = c % 4 of 4 heads).
Each core computes attention for its 4 heads of its batch and a partial
output projection [2048, 1024]; the host sums the 4 head-group partials
per batch and adds b_out plus the V-bias correction.

Engine roofline per core (fp16): PE ~137us of matmul stream, ScalarE
~143us of softmax exp ([128,1024] ACTIVATE per kchunk) -- the exp stream
is the pacing engine, so the kernel is organized to keep it saturated
and to keep the PE free of idle gaps >3.4us (HAM re-throttle):

  - inputs are host-prepacked so every DMA is contiguous per partition
    (large packets), split across the two HWDGE queues (sync + scalar)
    in first-use order; only K-pair0/Q-pair0 of strip 0 gate the first
    score matmul. Warmup matmuls on a ones tile keep the PE busy (and
    HAM-warm) through the input-DMA window.
  - groups run p-major: (s0..s3, pair0) then (s0..s3, pair1). All other
    projections (remaining K/Q strips, V chunks) are woven into k-loop
    slots as background PE tasks with deadlines ahead of first use,
    from slot 4 on (slots 0-3 belong to the deferred normalize below).
  - scoresT pair [kpos 128, qpos 1024] via two concurrent row-tiled
    cont-64 matmuls; exp as one ACT op per kchunk (bias = mask bias);
    AV lags one kchunk (lhsT = V chunk [128, 64+1 ones col] so the
    softmax denominator rides the AV matmul).
  - normalization of group g (denominator stash, approx reciprocal,
    fp16 1-partition PE outer-product broadcast, DVE multiply) runs as
    background units in slots 0-1 of group g+1, after that group's
    first scores are already in the PE queue -- group boundaries never
    head-of-line-block the in-order PE stream.
  - outproj for strip s runs as background tasks one full group after
    (s, pair1); y is written fp16 (partials summed on the host in fp32).
"""

import sys

sys.path.insert(0, "/opt/trn_rl_repo")

import numpy as np

B, N, D = 2, 2048, 1024
HEADS, DH = 16, 64
SCALE = DH ** -0.5
NCORES = 8
GROUPS = 4                      # head groups (tensor parallel)
DLOC = (HEADS // GROUPS) * DH   # 256 local inner dims per core

QC = 4                          # 512-wide query strips
DC = 8                          # contraction chunks
KC = 16                         # 128-wide key chunks
NWARM = 12                      # PE warmup matmuls during the DMA window

_CACHE = {}


def build_model(with_bias=False):
    """Build (once) the single-core Bass/Tile program shared by all 8 cores.

    with_bias adds the Q/K bias matmuls (b_qkv is all-zero per the problem
    spec, so the default model omits them)."""
    key = ("nc", with_bias)
    if key in _CACHE:
        return _CACHE[key]

    from concourse import bacc, mybir, tile

    f32 = mybir.dt.float32
    f16 = mybir.dt.float16
    AF = mybir.ActivationFunctionType

    nc = bacc.Bacc("TRN2", target_bir_lowering=False, debug=False)

    xt_d = nc.dram_tensor("xt", [128, QC, DC, 512], f16, kind="ExternalInput").ap()
    wqkv_d = nc.dram_tensor("wqkv", [128, 3, DC, DLOC], f16, kind="ExternalInput").ap()
    wout_d = nc.dram_tensor("wout", [128, 2, D], f16, kind="ExternalInput").ap()
    mb_d = nc.dram_tensor("mb", [128, KC], f32, kind="ExternalInput").ap()
    ones_d = nc.dram_tensor("ones16", [128, 512], f16, kind="ExternalInput").ap()
    brow_d = nc.dram_tensor("brow", [1, 3 * DLOC], f16, kind="ExternalInput").ap()
    y_d = nc.dram_tensor("y", [N, D], f16, kind="ExternalOutput").ap()

    with tile.TileContext(nc) as tc:
        with (
            tc.tile_pool(name="res", bufs=1) as res,
            tc.tile_pool(name="exp", bufs=8) as exp_pool,
            tc.tile_pool(name="ysb", bufs=3) as y_pool,
            tc.tile_pool(name="small", bufs=8) as small_pool,
            tc.tile_pool(name="ps", bufs=4, space="PSUM") as ps,
            tc.tile_pool(name="spair", bufs=2, space="PSUM") as ps_s,
        ):
            xt = res.tile([128, QC, DC, 512], f16)
            wqkv = res.tile([128, 3, DC, DLOC], f16)
            wout = res.tile([128, 2, D], f16)
            mb = res.tile([128, KC], f32)
            ones16 = res.tile([128, 512], f16)
            qt = res.tile([128, 2, N], f16)
            kt = res.tile([128, 2, N], f16)
            vn = res.tile([128, KC, 4, 65], f16)
            outt = res.tile([128, 2, N], f16)
            brow = res.tile([1, 3 * DLOC], f16) if with_bias else None

            # ---- input DMAs, all on the sync HWDGE queue (a DMA occupies
            # its issuing engine for the transfer, so the scalar engine must
            # stay clear for the exp stream), ordered by first-use deadline.
            nc.sync.dma_start(ones16[:], ones_d[:])
            nc.sync.dma_start(wqkv[:, 1:2], wqkv_d[:, 1:2])
            nc.sync.dma_start(xt[:, 0:1], xt_d[:, 0:1])
            nc.sync.dma_start(wqkv[:, 0:1], wqkv_d[:, 0:1])
            nc.sync.dma_start(mb[:], mb_d[:])
            nc.sync.dma_start(xt[:, 1:2], xt_d[:, 1:2])
            nc.sync.dma_start(wqkv[:, 2:3, :, 0:128], wqkv_d[:, 2:3, :, 0:128])
            nc.sync.dma_start(
                vn[:, :, :, 64:65],
                ones_d[:, 0:64].rearrange("p (j h) -> p j h", h=4).unsqueeze(-1),
            )
            nc.sync.dma_start(xt[:, 2:3], xt_d[:, 2:3])
            nc.sync.dma_start(wqkv[:, 2:3, :, 128:256], wqkv_d[:, 2:3, :, 128:256])
            nc.sync.dma_start(xt[:, 3:4], xt_d[:, 3:4])
            nc.sync.dma_start(wout[:], wout_d[:])
            if with_bias:
                nc.sync.dma_start(brow[:], brow_d[:])

            # ---- PE warmup through the DMA window: keeps HAM at 8/8 so the
            # first real matmuls run at 2.4 GHz.
            wps = ps.tile([128, 512], f32, tag="ps", name="warm")
            for _ in range(NWARM):
                nc.tensor.matmul(
                    wps[:], ones16[:, 0:128], ones16[:], start=True, stop=True
                )

            # ---- background PE task units (each <= ~0.7us of PE stream) ----
            def qk_mms(t, s, pr, psum, cs):
                """Score-side projection chunk matmuls (t: 0=Q, 1=K)."""
                for c in cs:
                    nc.tensor.matmul(
                        psum[:],
                        wqkv[:, t, c, pr * 128:(pr + 1) * 128],
                        xt[:, s, c, :],
                        start=(c == 0),
                        stop=(not with_bias and c == DC - 1),
                    )
                if with_bias and DC - 1 in cs:
                    col0 = t * DLOC + pr * 128
                    nc.tensor.matmul(
                        psum[:],
                        brow[0:1, col0:col0 + 128],
                        ones16[0:1, 0:512],
                        start=False,
                        stop=True,
                    )

            def proj_qk_units(t, s, pr, dst):
                """3 units: one strip/pair of the Q^T or K^T projection."""
                state = {}

                def u1():
                    state["ps"] = ps.tile([128, 512], f32, tag="ps", name="qkps")
                    qk_mms(t, s, pr, state["ps"], (0, 1, 2))

                def u2():
                    qk_mms(t, s, pr, state["ps"], (3, 4, 5))

                def u3():
                    qk_mms(t, s, pr, state["ps"], (6, 7))
                    nc.vector.tensor_copy(
                        dst[:, pr, s * 512:(s + 1) * 512], state["ps"][:]
                    )

                return [u1, u2, u3]

            def vproj_unit(j, pr):
                """V projection of seq chunk j for head pair pr (128 cols)."""

                def u():
                    psum = ps.tile([128, 128], f32, tag="ps", name="vps")
                    for c in range(DC):
                        nc.tensor.matmul(
                            psum[:],
                            xt[:, j // 4, c, (j % 4) * 128:(j % 4) * 128 + 128],
                            wqkv[:, 2, c, pr * 128:(pr + 1) * 128],
                            start=(c == 0),
                            stop=(c == DC - 1),
                        )
                    nc.vector.tensor_copy(
                        vn[:, j, 2 * pr:2 * pr + 2, 0:64],
                        psum[:].rearrange("a (h x) -> a h x", h=2),
                    )

                return u

            def outproj_tasks(s):
                """8 units: output projection of strip s as (jj, nb) groups."""
                state = {}
                tasks = []
                for jj in range(4):
                    for nb in range(2):
                        def t(jj=jj, nb=nb):
                            q0 = s * 512 + jj * 128
                            if nb == 0:
                                state[jj] = y_pool.tile(
                                    [128, D], f16, tag="ysb", name="ysb"
                                )
                            ysb = state[jj]
                            yps = ps.tile([128, 512], f32, tag="ps", name="yps")
                            for p_ in range(2):
                                nc.tensor.matmul(
                                    yps[:],
                                    outt[:, p_, q0:q0 + 128],
                                    wout[:, p_, nb * 512:(nb + 1) * 512],
                                    start=(p_ == 0),
                                    stop=(p_ == 1),
                                )
                            nc.vector.tensor_copy(
                                ysb[:, nb * 512:(nb + 1) * 512], yps[:]
                            )
                            if nb == 1:
                                nc.sync.dma_start(y_d[q0:q0 + 128, :], ysb[:])
                        tasks.append(t)
                return tasks

            def norm_units(s, p, av):
                """2 units: normalization of a finished group's AV psum pair.
                Each: stash denominator + unnormalized rows (frees av psum),
                approx reciprocal, fp16 cast, PE outer-product broadcast,
                DVE multiply into outT."""
                units = []
                for i in range(2):
                    def u(i=i):
                        dnr = small_pool.tile([1, 512], f32, tag="dnr", name="dnr")
                        nc.vector.tensor_copy(dnr[:], av[i][64:65, :])
                        un = small_pool.tile([64, 512], f32, tag="un", name="un")
                        nc.vector.tensor_copy(un[:], av[i][0:64, :])
                        rc = small_pool.tile([1, 512], f32, tag="rc", name="rc")
                        nc.vector.reciprocal_approx_fast(rc[:], dnr[:])
                        rch = small_pool.tile([1, 512], f16, tag="rch", name="rch")
                        nc.vector.tensor_copy(rch[:], rc[:])
                        bc = ps.tile([64, 512], f32, tag="ps", name="bc")
                        nc.tensor.matmul(
                            bc[:], ones16[0:1, 0:64], rch[0:1, :],
                            start=True, stop=True,
                        )
                        nc.vector.tensor_mul(
                            outt[64 * i:64 * i + 64, p, s * 512:(s + 1) * 512],
                            un[:],
                            bc[:],
                        )
                    units.append(u)
                return units

            def attn_group(s, p, slot_tasks):
                """One (strip, head-pair) attention group: 16 kchunk slots of
                scores pair -> exp -> lag-2 AV, with background units woven
                in. The last two AV pairs and the normalize run as deferred
                units in the next group's first slots (so a group boundary
                never head-of-line-blocks the PE queue)."""
                av = [None, None]
                exs = [None] * KC
                LAG = 2
                for k in range(KC):
                    sc = ps_s.tile([128, 1024], f32, tag="spair", name="sc")
                    for i in range(2):
                        nc.tensor.matmul(
                            sc[:, i * 512:(i + 1) * 512],
                            kt[64 * i:64 * i + 64, p, k * 128:(k + 1) * 128],
                            qt[64 * i:64 * i + 64, p, s * 512:(s + 1) * 512],
                            start=True,
                            stop=True,
                        )
                    ex = exp_pool.tile([128, 1024], f16, tag="exp", name="ex")
                    nc.scalar.activation(
                        ex[:], sc[:], AF.Exp, bias=mb[:, k:k + 1], scale=1.0
                    )
                    exs[k] = ex
                    if k >= LAG:
                        if k == LAG:
                            av[0] = ps.tile([65, 512], f32, tag="ps", name="av0")
                            av[1] = ps.tile([65, 512], f32, tag="ps", name="av1")
                        for i in range(2):
                            nc.tensor.matmul(
                                av[i][:],
                                vn[:, k - LAG, 2 * p + i, :],
                                exs[k - LAG][:, i * 512:(i + 1) * 512],
                                start=(k - LAG == 0),
                                stop=False,
                            )
                    for u in slot_tasks.get(k, ()):
                        u()

                def fin_av():
                    for kk in range(KC - LAG, KC):
                        for i in range(2):
                            nc.tensor.matmul(
                                av[i][:],
                                vn[:, kk, 2 * p + i, :],
                                exs[kk][:, i * 512:(i + 1) * 512],
                                start=False,
                                stop=(kk == KC - 1),
                            )
                return [fin_av] + norm_units(s, p, av)

            def sched(assignments):
                """{slot: [units...]} from a list of (slot, unit)."""
                d = {}
                for sl, u in assignments:
                    d.setdefault(sl, []).append(u)
                return d

            # ---- phase 0: minimal pre-attention (K and Q strip 0, pair 0)
            for u in proj_qk_units(1, 0, 0, kt):
                u()
            for u in proj_qk_units(0, 0, 0, qt):
                u()

            # ---- group schedule (p-major) with background weave.
            # slots 0-1 of each group carry the previous group's normalize;
            # other units start at slot 2/4. deadlines: K(s,0) by group0 slot
            # 4s; vp(j,0) by group0 slot j+1; K/Q/v pair1 before group 4(+s).
            asg = {g: [] for g in range(8)}
            for j in range(KC):
                asg[0].append((j, vproj_unit(j, 0)))
            for si, base in ((1, 0), (2, 4), (3, 8)):
                for ui, u in enumerate(proj_qk_units(1, si, 0, kt)):
                    asg[0].append((base + ui, u))
            for ui, u in enumerate(proj_qk_units(0, 1, 0, qt)):
                asg[0].append((12 + ui, u))

            g1 = (proj_qk_units(0, 2, 0, qt) + proj_qk_units(1, 0, 1, kt)
                  + [vproj_unit(j, 1) for j in range(0, 6)])
            g2 = (proj_qk_units(0, 3, 0, qt) + proj_qk_units(1, 1, 1, kt)
                  + [vproj_unit(j, 1) for j in range(6, 12)])
            g3 = (proj_qk_units(0, 0, 1, qt) + proj_qk_units(1, 2, 1, kt)
                  + [vproj_unit(j, 1) for j in range(12, 16)])
            g4 = proj_qk_units(1, 3, 1, kt) + proj_qk_units(0, 1, 1, qt)
            g5 = proj_qk_units(0, 2, 1, qt) + outproj_tasks(0)
            g6 = proj_qk_units(0, 3, 1, qt) + outproj_tasks(1)
            g7 = outproj_tasks(2)
            for g, units in ((1, g1), (2, g2), (3, g3), (4, g4),
                             (5, g5), (6, g6), (7, g7)):
                for ui, u in enumerate(units):
                    asg[g].append((3 + ui, u))

            order = [(s, 0) for s in range(QC)] + [(s, 1) for s in range(QC)]
            pending = []
            for g, (s, p) in enumerate(order):
                slot_tasks = sched(
                    [(i, u) for i, u in enumerate(pending)] + asg[g]
                )
                pending = attn_group(s, p, slot_tasks)
            for u in pending:
                u()
            for t in outproj_tasks(3):
                t()

    nc.compile()
    _CACHE[key] = nc
    return nc


def make_in_maps(x, mask, W_qkv, b_qkv, W_out):
    x = np.asarray(x, np.float32)
    W_qkv = np.asarray(W_qkv, np.float32)
    b_qkv = np.asarray(b_qkv, np.float32)
    W_out = np.asarray(W_out, np.float32)
    if mask is None:
        m = np.ones((B, N), bool)
    else:
        mask = np.asarray(mask, bool)
        m = np.concatenate([np.ones((B, 1), bool), mask], axis=1)
    mbias = np.where(m, np.float32(0.0), np.float32(-1e30)).astype(np.float32)

    in_maps = []
    for c in range(NCORES):
        b, g = divmod(c, GROUPS)
        cs = slice(DLOC * g, DLOC * g + DLOC)
        wq = W_qkv[:, 0:D][:, cs] * SCALE
        wk = W_qkv[:, D:2 * D][:, cs]
        wv = W_qkv[:, 2 * D:3 * D][:, cs]
        bq = b_qkv[0:D][cs] * SCALE
        bk = b_qkv[D:2 * D][cs]
        bv = np.zeros(DLOC, np.float32)   # V bias applied in combine()
        # xt[p, s, c, n] = x[b, s*512+n, c*128+p]
        xt = x[b].reshape(QC, 512, DC, 128).transpose(3, 0, 2, 1)
        # wqkv[p, t, c, j] = W_t[c*128+p, j]
        wqkv = np.stack(
            [w.reshape(DC, 128, DLOC).transpose(1, 0, 2) for w in (wq, wk, wv)],
            axis=1,
        )
        in_maps.append({
            "xt": np.ascontiguousarray(xt).astype(np.float16),
            "wqkv": np.ascontiguousarray(wqkv).astype(np.float16),
            "wout": np.ascontiguousarray(
                W_out[cs, :].reshape(2, 128, D).transpose(1, 0, 2)
            ).astype(np.float16),
            "mb": np.ascontiguousarray(mbias[b].reshape(KC, 128).T),
            "ones16": np.ones((128, 512), np.float16),
            "brow": np.concatenate([bq, bk, bv])[None, :].astype(np.float16),
        })
    return in_maps


def combine(results, b_qkv, W_out, b_out):
    out = np.zeros((B, N, D), np.float32)
    for c in range(NCORES):
        out[c // GROUPS] += np.asarray(results[c]["y"], np.float32)
    b_qkv = np.asarray(b_qkv, np.float32)
    W_out = np.asarray(W_out, np.float32)
    # attention rows sum to 1 -> V bias contributes b_v @ W_out everywhere
    out += (b_qkv[2 * D:3 * D] @ W_out)[None, None, :]
    out += np.asarray(b_out, np.float32)[None, None, :]
    return out


def kernel(x, mask=None, W_qkv=None, b_qkv=None, W_out=None, b_out=None, **kw):
    from concourse.bass_utils import run_bass_kernel_spmd

    qk_bias = np.any(np.asarray(b_qkv, np.float32)[0:2 * D])
    nc = build_model(with_bias=bool(qk_bias))
    in_maps = make_in_maps(x, mask, W_qkv, b_qkv, W_out)
    res = run_bass_kernel_spmd(nc, in_maps, core_ids=list(range(NCORES)))
    return combine(res.results, b_qkv, W_out, b_out)
